# revision 24
# baseline (speedup 1.0000x reference)
"""Distributed Bass kernel for nn_Attention (B=2, T=2048, D=1024, H=16) on 8 TRN2 cores.

Sharding: core c -> (batch b = c//4, head-group g = c%4, heads 4g..4g+3).
QKV tensor-parallel over heads, out-proj row-parallel + ReduceScatter(4-rank groups).
"""

import functools
import numpy as np
from contextlib import ExitStack

B, T, D, H, HD = 2, 2048, 1024, 16, 64
EPS = 1e-4
NCORES, GROUP = 8, 4
HL = H // GROUP          # heads per core = 4
DL = HL * HD             # local feature cols = 256
NTT = T // 128           # 16 token tiles
NDT = D // 128           # 8 d tiles
NWT = (3 * DL) // 128    # 6 w_qkv row tiles
NIKB = T // 1024         # 2 query kilo-blocks


def _build_bass():
    import concourse.bass as bass
    import concourse.tile as tile
    from concourse import bacc, mybir

    f32 = mybir.dt.float32
    f32r = mybir.dt.float32r
    bf16 = mybir.dt.bfloat16
    AX = mybir.AxisListType
    OP = mybir.AluOpType
    AF = mybir.ActivationFunctionType

    nc = bacc.Bacc("TRN2", target_bir_lowering=False, debug=False, num_devices=NCORES)

    xT_ext = nc.dram_tensor("xT", [D, T], f32, kind="ExternalInput").ap()
    wqkv_ext = nc.dram_tensor("wqkv", [3 * DL, D], f32, kind="ExternalInput").ap()
    woutT_ext = nc.dram_tensor("woutT", [DL, D], f32, kind="ExternalInput").ap()
    wout_ext = nc.dram_tensor("wout", [D, D], f32, kind="ExternalInput").ap()
    out_ext = nc.dram_tensor("out", [DL, T], f32, kind="ExternalOutput").ap()

    import ml_dtypes
    ident_np = np.eye(128, dtype=ml_dtypes.bfloat16)
    ones_np = np.ones((1, 64), dtype=np.float32)

    with tile.TileContext(nc) as tc, ExitStack() as ctx:
        # ---------------- persistent pools ----------------
        pers = ctx.enter_context(tc.tile_pool(name="pers", bufs=1))
        dram = ctx.enter_context(tc.tile_pool(name="dram", bufs=1, space="DRAM"))

        id_sb = pers.tile([128, 128], bf16)
        ident_dram = nc.inline_tensor(ident_np, name="ident_c")
        nc.sync.dma_start(id_sb[:], ident_dram.ap())

        xT_sb = pers.tile([128, NDT, T], bf16)
        whT_sb = pers.tile([128, NDT, 3 * DL], bf16)
        WT_sb = pers.tile([128, 2, D], bf16)
        qT_sb = [pers.tile([128, T], bf16, name=f"qT{rb}") for rb in range(2)]
        kT_sb = [pers.tile([128, T], bf16, name=f"kT{rb}") for rb in range(2)]
        # per-head duplicated layouts (head h at partitions 0-63 AND 64-127)
        # for 2x row-tiled score matmuls on j-parity
        qTd = [pers.tile([128, T], bf16, name=f"qTd{h}") for h in range(HL)]
        kTd = [pers.tile([128, T], bf16, name=f"kTd{h}") for h in range(HL)]
        v_sb = pers.tile([128, NTT, HL * 65], bf16)
        b_all = pers.tile([128, NTT, HL], f32)
        s_out = pers.tile([128, NDT], f32)
        aoT_sb = [pers.tile([128, T], bf16, name=f"aoT{rb}") for rb in range(2)]

        NRS = 4  # ReduceScatter chunks over tokens
        qnat = dram.tile([T, DL], bf16)
        knat = dram.tile([T, DL], bf16)
        rs_in = [dram.tile([D, T // NRS], bf16, name=f"rs_in{k}")
                 for k in range(NRS)]
        rs_out = [dram.tile([DL, T // NRS], bf16, name=f"rs_out{k}")
                  for k in range(NRS)]

        # ---------------- input DMAs (ordered: wqkv, x, then late weights) ----
        with tc.tile_pool(name="wphase", bufs=1) as wp, \
             tc.tile_pool(name="wpsum", bufs=2, space="PSUM") as wps:
            w_sb = wp.tile([128, NWT, D], bf16)
            wo_sb = wp.tile([128, NDT, D], bf16)
            nc.gpsimd.dma_start(w_sb[:], wqkv_ext.rearrange("(n p) d -> p n d", p=128))
            for half in range(2):
                nc.gpsimd.dma_start(
                    xT_sb[:, 4 * half : 4 * (half + 1), :],
                    xT_ext.rearrange("(n p) t -> p n t", p=128)[:, 4 * half : 4 * (half + 1), :],
                )
            nc.gpsimd.dma_start(WT_sb[:], woutT_ext.rearrange("(n p) d -> p n d", p=128))
            nc.gpsimd.dma_start(wo_sb[:], wout_ext.rearrange("(n p) d -> p n d", p=128))
            nc.gpsimd.memset(v_sb[:], 1.0)

            # ---------------- phase W: normalize weights ----------------
            sq_scr = wp.tile([128, D], bf16)
            n2w = wp.tile([128, NWT], f32)
            n2o = wp.tile([128, NDT], f32)
            for n in range(NWT):
                nc.scalar.activation(sq_scr[:], w_sb[:, n, :], AF.Square,
                                     accum_out=n2w[:, n : n + 1])
            # s = 1/(norm + 32*eps)  [w_hat = w / (eps + norm/32) / 32]
            s_w = wp.tile([128, NWT], f32)
            nc.scalar.sqrt(n2w[:], n2w[:])
            nc.vector.tensor_scalar_add(n2w[:], n2w[:], 32.0 * EPS)
            nc.vector.reciprocal(s_w[:], n2w[:])

            what = wp.tile([128, NWT, D], bf16)
            for n in range(NWT):
                nc.vector.tensor_scalar_mul(what[:, n, :], w_sb[:, n, :],
                                            s_w[:, n : n + 1])
            # transpose w_hat [768, 1024] -> whT [1024, 768] via PE
            for n in range(NWT):
                for dt_ in range(NDT):
                    tp = wps.tile([128, 128], bf16)
                    nc.tensor.transpose(
                        tp[:], what[:, n, 128 * dt_ : 128 * (dt_ + 1)], id_sb[:])
                    nc.vector.tensor_copy(
                        whT_sb[:, dt_, 128 * n : 128 * (n + 1)], tp[:])

            # w_out row norms (needed only at out-proj eviction)
            for n in range(NDT):
                nc.scalar.activation(sq_scr[:], wo_sb[:, n, :], AF.Square,
                                     accum_out=n2o[:, n : n + 1])
            nc.scalar.sqrt(n2o[:], n2o[:])
            nc.vector.tensor_scalar_add(n2o[:], n2o[:], 32.0 * EPS)
            nc.vector.reciprocal(s_out[:], n2o[:])

        # ---------------- phase QKV ----------------
        with tc.tile_pool(name="qkvps", bufs=2, space="PSUM") as qps, \
             tc.tile_pool(name="qkvsb", bufs=3) as qsb:
            for tt in range(NTT):
                ps = qps.tile([128, 3 * DL], f32)
                for dt_ in range(NDT):
                    lhsT = xT_sb[:, dt_, 128 * tt : 128 * (tt + 1)]
                    nc.tensor.matmul(ps[:, 0:512], lhsT, whT_sb[:, dt_, 0:512],
                                     start=(dt_ == 0), stop=(dt_ == NDT - 1))
                    nc.tensor.matmul(ps[:, 512:768], lhsT, whT_sb[:, dt_, 512:768],
                                     start=(dt_ == 0), stop=(dt_ == NDT - 1))
                # evict q+k raw to sbuf, then norms from the sbuf copy
                qk_raw = qsb.tile([128, 2 * DL], bf16)
                nc.scalar.copy(qk_raw[:], ps[:, 0 : 2 * DL])
                nc.scalar.activation(
                    v_sb[:, tt, :].rearrange("p (h c) -> p h c", c=65)[:, :, 0:HD],
                    ps[:, 2 * DL : 3 * DL].rearrange("p (h c) -> p h c", c=HD),
                    AF.Copy)
                sq = qsb.tile([128, 2 * DL], f32)
                nc.vector.tensor_tensor(sq[:], qk_raw[:], qk_raw[:], op=OP.mult)
                n2 = qsb.tile([128, 2 * HL], f32)
                nc.vector.reduce_sum(
                    n2[:], sq[:].rearrange("p (h c) -> p h c", c=HD), axis=AX.X)
                nc.scalar.sqrt(n2[:], n2[:])
                nc.vector.tensor_scalar_add(n2[:], n2[:], 8.0 * EPS)
                a_q = qsb.tile([128, HL], f32)
                nc.vector.reciprocal(a_q[:], n2[:, 0:HL])
                nc.vector.reciprocal(b_all[:, tt, :], n2[:, HL : 2 * HL])
                # scale q by 8*a (per head), k stays raw
                qst = qsb.tile([128, DL], bf16)
                for h in range(HL):
                    nc.vector.tensor_scalar(
                        qst[:, HD * h : HD * (h + 1)],
                        qk_raw[:, HD * h : HD * (h + 1)],
                        a_q[:, h : h + 1], 8.0, op0=OP.mult, op1=OP.mult)
                nc.sync.dma_start(qnat[128 * tt : 128 * (tt + 1), :], qst[:])
                nc.sync.dma_start(knat[128 * tt : 128 * (tt + 1), :],
                                  qk_raw[:, DL : 2 * DL])

        # transpose q,k: [T, 256] -> 2x [128, T] (heads 2rb, 2rb+1)
        for rb in range(2):
            nc.sync.dma_start_transpose(qT_sb[rb][:], qnat[:, 128 * rb : 128 * (rb + 1)])
            nc.sync.dma_start_transpose(kT_sb[rb][:], knat[:, 128 * rb : 128 * (rb + 1)])
        # duplicate each head into both partition halves (for row-tiled pairs)
        for h in range(HL):
            rb, hh = h // 2, h % 2
            for half in range(2):
                nc.gpsimd.tensor_copy(qTd[h][64 * half : 64 * (half + 1), :],
                                 qT_sb[rb][64 * hh : 64 * (hh + 1), :])
                nc.gpsimd.tensor_copy(kTd[h][64 * half : 64 * (half + 1), :],
                                 kT_sb[rb][64 * hh : 64 * (hh + 1), :])

        # ---------------- ATTN (ikb-outer) + overlapped OUTPROJ/RS ----------
        with tc.tile_pool(name="scps", bufs=2, space="PSUM") as scps, \
             tc.tile_pool(name="atps", bufs=2, space="PSUM") as atps, \
             tc.tile_pool(name="exsb", bufs=4) as exsb, \
             tc.tile_pool(name="rssb", bufs=2) as rssb, \
             tc.tile_pool(name="ysb", bufs=2) as ysb:

            def rs_dance(po, h, ikb):
                """divide outT rows 0..63 by rowsum row 64, write into aoT."""
                rb, hh = h // 2, h % 2
                rsum = rssb.tile([1, 1024], f32, name="rsum")
                nc.vector.tensor_copy(rsum[:], po[64:65, :])
                rinv = rssb.tile([1, 1024], f32, name="rinv")
                nc.vector.reciprocal_approx_fast(rinv[:], rsum[:])
                bc_sb = rssb.tile([64, 1024], f32, name="bc_sb")
                nc.gpsimd.partition_broadcast(bc_sb[:], rinv[:])
                nc.vector.tensor_tensor(
                    aoT_sb[rb][64 * hh : 64 * (hh + 1),
                               1024 * ikb : 1024 * (ikb + 1)],
                    po[0:64, :], bc_sb[:], op=OP.mult)

            def outproj_rs(krs):
                """out-proj for token chunk krs (512 wide) + its ReduceScatter."""
                for dt_ in range(NDT):
                    yst = ysb.tile([128, 512], bf16, name="yst")
                    yp = atps.tile([128, 512], f32, name="yp", tag="po")
                    for ft in range(2):
                        nc.tensor.matmul(
                            yp[:], WT_sb[:, ft, 128 * dt_ : 128 * (dt_ + 1)],
                            aoT_sb[ft][:, 512 * krs : 512 * (krs + 1)],
                            start=(ft == 0), stop=(ft == 1))
                    nc.vector.tensor_scalar_mul(yst[:], yp[:],
                                                s_out[:, dt_ : dt_ + 1])
                    nc.sync.dma_start(
                        rs_in[krs][128 * dt_ : 128 * (dt_ + 1), :], yst[:])
                nc.gpsimd.collective_compute(
                    "ReduceScatter", mybir.AluOpType.add,
                    replica_groups=[[0, 1, 2, 3], [4, 5, 6, 7]],
                    ins=[rs_in[krs].opt()], outs=[rs_out[krs].opt()])
                nc.gpsimd.dma_start(
                    out_ext[:, 512 * krs : 512 * (krs + 1)], rs_out[krs][:])

            for ikb in range(NIKB):
                pending = None
                for h in range(HL):
                    po = atps.tile([65, 1024], f32, name="po")
                    for j in range(NTT):
                        par = j % 2  # partition half: row-tiled j-parity pairs
                        ksl = kTd[h][64 * par : 64 * (par + 1),
                                     128 * j : 128 * (j + 1)]
                        qsl = qTd[h][64 * par : 64 * (par + 1),
                                     1024 * ikb : 1024 * (ikb + 1)]
                        sc = scps.tile([128, 1024], f32, name="sc", tag="sc")
                        for half in range(2):
                            nc.tensor.matmul(
                                sc[:, 512 * half : 512 * (half + 1)], ksl,
                                qsl[:, 512 * half : 512 * (half + 1)],
                                start=True, stop=True)
                        ex = exsb.tile([128, 1024], bf16, name="ex")
                        nc.scalar.activation(ex[:], sc[:], AF.Exp,
                                             scale=b_all[:, j, h : h + 1])
                        for half in range(2):
                            nc.tensor.matmul(
                                po[:, 512 * half : 512 * (half + 1)],
                                v_sb[:, j, 65 * h : 65 * (h + 1)],
                                ex[:, 512 * half : 512 * (half + 1)],
                                start=(j == 0), stop=(j == NTT - 1))
                    if pending is not None:
                        rs_dance(*pending)
                    pending = (po, h, ikb)
                rs_dance(*pending)
                outproj_rs(2 * ikb)
                outproj_rs(2 * ikb + 1)

    nc.compile()
    return nc


@functools.lru_cache(maxsize=1)
def _get_nc():
    return _build_bass()


def kernel(x: np.ndarray, w_qkv: np.ndarray, w_out: np.ndarray) -> np.ndarray:
    from concourse.bass_utils import run_bass_kernel_spmd

    x = np.asarray(x, dtype=np.float32)
    w_qkv = np.asarray(w_qkv, dtype=np.float32)
    w_out = np.asarray(w_out, dtype=np.float32)

    woutT = np.ascontiguousarray(w_out.T)
    in_maps = []
    for c in range(NCORES):
        b, g = c // GROUP, c % GROUP
        rows = np.concatenate([
            np.arange(DL * g, DL * (g + 1)),
            D + np.arange(DL * g, DL * (g + 1)),
            2 * D + np.arange(DL * g, DL * (g + 1)),
        ])
        in_maps.append({
            "xT": np.ascontiguousarray(x[b].T),
            "wqkv": np.ascontiguousarray(w_qkv[rows]),
            "woutT": np.ascontiguousarray(woutT[DL * g : DL * (g + 1)]),
            "wout": w_out,
        })

    nc = _get_nc()
    res = run_bass_kernel_spmd(nc, in_maps, core_ids=list(range(NCORES)))

    out = np.empty((B, T, D), dtype=np.float32)
    for c in range(NCORES):
        b, g = c // GROUP, c % GROUP
        out[b][:, DL * g : DL * (g + 1)] = res.results[c]["out"].T
    return out
